# revision 7
# baseline (speedup 1.0000x reference)
"""Multi-head self-attention (d_model=1024, 16 heads, b=2, n=2048) on 8 TRN2 NeuronCores.

Sharding: tensor-parallel over heads (2 heads = 128 projection dims per core).
Each core computes Q^T/K^T/V for its head slice over all 4096 rows, runs
attention in the transposed (scores^T = [k, q]) layout so no on-chip
transposes are needed inside the attention loop, then an AllToAll converts
head-sharding to row-sharding for the output projection (contraction over all
1024 dims, 512 rows per core).

Layout notes (per core):
  - x^T [1024, 4096] is produced by PE transpose-mode matmuls from streamed
    256-row chunks of x.
  - Q^T/K^T [128 dims, 4096 rows]; K^T has bias and 1/sqrt(64) folded in,
    Q^T has bias folded in.
  - V natural [4096 rows, 128 dims] (32 SBUF tiles of [128, 128]).
  - scores^T tile (kt, q-chunk) = lhsT(K^T[d, k-tile]).T @ rhs(Q^T[d, q-chunk]),
    row-packed pairs: head A on array rows 0-63, head B on rows 64-127.
  - softmax denominators via col-packed all-ones matmuls -> partition-broadcast
    sums in PSUM [128, 512]; exp() runs on ACT straight out of PSUM.
  - out^T [dims, q] = lhsT(V[k, d]).T @ rhs(exp^T[k, q]), col-packed heads;
    normalized by one reciprocal + one multiply on DVE.
  - o-proj: lhsT = gathered out^T [ctile, rows], rhs = wo[ctile, :]; bias bo
    added with a K=1 all-ones matmul.
"""

import numpy as np

import concourse.bass as bass
import concourse.mybir as mybir
import concourse.tile as tile
from concourse import bacc, bass_utils
from concourse.masks import make_identity

N_CORES = 8
D = 1024            # d_model
ROWS = 4096         # b*n
NSEQ = 2048         # seq len per batch
B = 2
HD = 128            # head-dims per core (2 heads x 64)
RC = 256            # x streaming chunk (rows)
N_RC = ROWS // RC   # 16
KT = 128            # key tile
N_KT = NSEQ // KT   # 16 per batch
QC = 512            # query chunk
N_QC = NSEQ // QC   # 4 per batch
R_OUT = ROWS // N_CORES  # 512 output rows per core

f32 = mybir.dt.float32
f32r = mybir.dt.float32r

_LAST_RESULTS = None  # BassKernelResults from the most recent run (for test.py)


def _r(ap):
    return ap.bitcast(f32r)


def build_program():
    nc = bacc.Bacc("TRN2", target_bir_lowering=False, debug=False,
                   num_devices=N_CORES)

    x = nc.dram_tensor("x", [ROWS, D], f32, kind="ExternalInput")
    wq = nc.dram_tensor("wq", [D, HD], f32, kind="ExternalInput")
    wk = nc.dram_tensor("wk", [D, HD], f32, kind="ExternalInput")
    wv = nc.dram_tensor("wv", [D, HD], f32, kind="ExternalInput")
    bq = nc.dram_tensor("bq", [HD, 1], f32, kind="ExternalInput")
    bk = nc.dram_tensor("bk", [HD, 1], f32, kind="ExternalInput")
    bv = nc.dram_tensor("bv", [HD, 1], f32, kind="ExternalInput")
    wo = nc.dram_tensor("wo", [D, D], f32, kind="ExternalInput")
    bo = nc.dram_tensor("bo", [1, D], f32, kind="ExternalInput")
    y = nc.dram_tensor("y", [R_OUT, D], f32, kind="ExternalOutput")

    scale = 1.0 / 8.0  # 1/sqrt(64)

    with tile.TileContext(nc) as tc:
        with (
            tc.tile_pool(name="const", bufs=1) as cpool,
            tc.tile_pool(name="qkv", bufs=1) as qkvpool,
            tc.tile_pool(name="dram", bufs=1, space="DRAM") as dpool,
        ):
            ident_f = cpool.tile([128, 128], f32)
            make_identity(nc, ident_f[:])
            ident = cpool.tile([128, 128], f32r)
            nc.vector.tensor_copy(ident[:], ident_f[:])
            ones_f = cpool.tile([128, 64], f32)
            nc.vector.memset(ones_f[:], 1.0)
            ones64 = cpool.tile([128, 64], f32r)
            nc.vector.tensor_copy(ones64[:], ones_f[:])
            ones1_f = cpool.tile([1, 128], f32)
            nc.vector.memset(ones1_f[:], 1.0)
            ones1 = cpool.tile([1, 128], f32r)
            nc.vector.tensor_copy(ones1[:], ones1_f[:])
            bq_sb = cpool.tile([HD, 1], f32)
            bk_sb = cpool.tile([HD, 1], f32)
            bv_sb = cpool.tile([HD, 1], f32)
            nc.sync.dma_start(bq_sb[:], bq[:])
            nc.sync.dma_start(bk_sb[:], bk[:])
            nc.sync.dma_start(bv_sb[:], bv[:])
            bo_sb = cpool.tile([1, D], f32r)
            nc.sync.dma_start(bo_sb[:], bo[:].bitcast(f32r))

            # persistent activations
            qT = qkvpool.tile([128, ROWS], f32r)   # [dims, rows]
            kT = qkvpool.tile([128, ROWS], f32r)   # [dims, rows], scaled
            # augmented V per head: 32 tiles of [128 rows, 64 V-dims | 64 ones]
            vA_sb = qkvpool.tile([128, 32 * 128], f32r)
            vB_sb = qkvpool.tile([128, 32 * 128], f32r)

            a2a_in = dpool.tile([D, R_OUT], f32)
            a2a_out = dpool.tile([D, R_OUT], f32)

            # ---------------- Phase A/B: x^T + projections ----------------
            with (
                tc.tile_pool(name="w", bufs=1) as wpool,
                tc.tile_pool(name="xin", bufs=2) as xpool,
                tc.tile_pool(name="xT", bufs=2) as xTpool,
                tc.tile_pool(name="vTc", bufs=2) as vTpool,
                tc.tile_pool(name="tpsum", bufs=4, space="PSUM") as tpsum,
                tc.tile_pool(name="ppsum", bufs=3, space="PSUM") as ppsum,
            ):
                # weights: [128, 8*128], k-tile t at free offset 128*t
                wq_sb = wpool.tile([128, 8 * HD], f32r)
                wk_sb = wpool.tile([128, 8 * HD], f32r)
                wv_sb = wpool.tile([128, 8 * HD], f32r)
                nc.sync.dma_start(wq_sb[:], wq.rearrange("(t p) h -> p t h", p=128).bitcast(f32r))
                nc.sync.dma_start(wk_sb[:], wk.rearrange("(t p) h -> p t h", p=128).bitcast(f32r))
                nc.sync.dma_start(wv_sb[:], wv.rearrange("(t p) h -> p t h", p=128).bitcast(f32r))

                for rc in range(N_RC):
                    # load x rows [rc*RC, (rc+1)*RC) as [128, 2*1024]
                    x_in = xpool.tile([128, 2 * D], f32r, tag="xin")
                    nc.sync.dma_start(
                        x_in[:],
                        x[rc * RC:(rc + 1) * RC, :].rearrange(
                            "(j p) d -> p j d", p=128).bitcast(f32r),
                    )
                    # transpose to xTc [1024 (8 tiles), 256]: tile k at free 256*k
                    xTc = xTpool.tile([128, 8 * RC], f32r, tag="xT")
                    for j in range(2):
                        for k in range(8):
                            tp = tpsum.tile([128, 128], f32, tag="tp")
                            nc.tensor.transpose(
                                _r(tp[:]),
                                x_in[:, j * D + k * 128: j * D + (k + 1) * 128],
                                ident[:],
                            )
                            nc.vector.tensor_copy(
                                xTc[:, k * RC + j * 128: k * RC + j * 128 + 128],
                                tp[:])

                    # projections for this chunk
                    for w_sb, b_sb, kind in (
                        (wq_sb, bq_sb, "q"),
                        (wk_sb, bk_sb, "k"),
                        (wv_sb, bv_sb, "v"),
                    ):
                        pp = ppsum.tile([128, RC], f32, tag="pp")
                        for k in range(8):
                            nc.tensor.matmul(
                                pp[:],
                                lhsT=w_sb[:, k * HD:(k + 1) * HD],
                                rhs=xTc[:, k * RC:(k + 1) * RC],
                                start=(k == 0),
                                stop=(k == 7),
                            )
                        if kind == "q":
                            nc.vector.tensor_scalar_add(
                                qT[:, rc * RC:(rc + 1) * RC], pp[:], b_sb[:])
                        elif kind == "k":
                            # (pp + bk) * scale
                            nc.vector.tensor_scalar(
                                kT[:, rc * RC:(rc + 1) * RC], pp[:],
                                b_sb[:], scale,
                                op0=mybir.AluOpType.add,
                                op1=mybir.AluOpType.mult,
                            )
                        else:
                            vTc = vTpool.tile([128, RC], f32r, tag="vTc")
                            nc.vector.tensor_scalar_add(vTc[:], pp[:], b_sb[:])
                            # transpose to V natural [rows, dims]
                            for j in range(2):
                                vp = tpsum.tile([128, 128], f32, tag="tp")
                                nc.tensor.transpose(
                                    _r(vp[:]),
                                    vTc[:, j * 128:(j + 1) * 128],
                                    ident[:],
                                )
                                rt = rc * 2 + j
                                nc.vector.tensor_copy(
                                    vA_sb[:, rt * 128: rt * 128 + 64],
                                    vp[:, 0:64])
                                nc.vector.tensor_copy(
                                    vB_sb[:, rt * 128: rt * 128 + 64],
                                    vp[:, 64:128])
                                nc.vector.tensor_copy(
                                    vA_sb[:, rt * 128 + 64: rt * 128 + 128],
                                    ones_f[:])
                                nc.vector.tensor_copy(
                                    vB_sb[:, rt * 128 + 64: rt * 128 + 128],
                                    ones_f[:])

            # ---------------- Phase C: attention ----------------
            # per (b, qc): scores^T in groups of 3 k-tiles -> exp (N=1536) ->
            # row-paired (K=64) matmuls with [V_h | ones] stationary: psum rows
            # 0-63 = unnormalized out^T, rows 64-127 = broadcast softmax sums.
            GK = 3  # k-tiles per exp group
            groups = [(g * GK, min(N_KT, (g + 1) * GK)) for g in range((N_KT + GK - 1) // GK)]
            with (
                tc.tile_pool(name="attn", bufs=10) as apool,
                tc.tile_pool(name="misc", bufs=6) as mpool,
                tc.tile_pool(name="spsum", bufs=2, space="PSUM") as spsum,
                tc.tile_pool(name="ph2", bufs=2, space="PSUM") as ph2_pool,
            ):
                for b in range(B):
                    for qc in range(N_QC):
                        q_off = b * NSEQ + qc * QC
                        eAs, eBs = [], []
                        for g0, g1 in groups:
                            gw = (g1 - g0) * QC
                            psA = spsum.tile([128, GK * QC], f32, tag="sc")
                            psB = spsum.tile([128, GK * QC], f32, tag="sc")
                            for kt in range(g0, g1):
                                i = kt - g0
                                k_off = b * NSEQ + kt * KT
                                nc.tensor.matmul(
                                    psA[:, i * QC:(i + 1) * QC],
                                    lhsT=kT[0:64, k_off:k_off + KT],
                                    rhs=qT[0:64, q_off:q_off + QC],
                                    start=True, stop=True,
                                    tile_position=(0, 0),
                                )
                                nc.tensor.matmul(
                                    psB[:, i * QC:(i + 1) * QC],
                                    lhsT=kT[64:128, k_off:k_off + KT],
                                    rhs=qT[64:128, q_off:q_off + QC],
                                    start=True, stop=True,
                                    tile_position=(64, 0),
                                )
                            eA = apool.tile([128, GK * QC], f32r, tag="attn")
                            eB = apool.tile([128, GK * QC], f32r, tag="attn")
                            nc.scalar.activation(
                                eA[:, 0:gw], psA[:, 0:gw],
                                mybir.ActivationFunctionType.Exp)
                            nc.scalar.activation(
                                eB[:, 0:gw], psB[:, 0:gw],
                                mybir.ActivationFunctionType.Exp)
                            eAs.append(eA)
                            eBs.append(eB)
                        gidx = b * N_QC + qc
                        for head, (vh_sb, ehs) in enumerate(
                                ((vA_sb, eAs), (vB_sb, eBs))):
                            psE = ph2_pool.tile([128, QC], f32, tag="ph2")
                            psO = ph2_pool.tile([128, QC], f32, tag="ph2")
                            for kt in range(N_KT):
                                vt = b * N_KT + kt
                                e_t = ehs[kt // GK]
                                i = kt % GK
                                first = kt == 0
                                last = kt == N_KT - 1
                                nc.tensor.matmul(
                                    psE[:],
                                    lhsT=vh_sb[0:64, vt * 128:(vt + 1) * 128],
                                    rhs=e_t[0:64, i * QC:(i + 1) * QC],
                                    start=first, stop=last,
                                    tile_position=(0, 0),
                                )
                                nc.tensor.matmul(
                                    psO[:],
                                    lhsT=vh_sb[64:128, vt * 128:(vt + 1) * 128],
                                    rhs=e_t[64:128, i * QC:(i + 1) * QC],
                                    start=first, stop=last,
                                    tile_position=(64, 0),
                                )
                            s_t = mpool.tile([128, QC], f32, tag="s")
                            nc.vector.tensor_copy(s_t[:], psE[:])
                            nc.vector.tensor_add(s_t[:], s_t[:], psO[:])
                            inv = mpool.tile([64, QC], f32, tag="inv")
                            nc.vector.reciprocal(inv[:], s_t[64:128, :])
                            outT = mpool.tile([64, QC], f32, tag="outT")
                            nc.vector.tensor_mul(outT[:], s_t[0:64, :], inv[:])
                            nc.sync.dma_start(
                                a2a_in[gidx * 128 + head * 64:
                                       gidx * 128 + head * 64 + 64, :],
                                outT[:])

            # ---------------- Phase D: A2A + o-proj ----------------
            nc.gpsimd.collective_compute(
                "AllToAll",
                mybir.AluOpType.bypass,
                replica_groups=[list(range(N_CORES))],
                ins=[a2a_in.opt()],
                outs=[a2a_out.opt()],
            )
            with (
                tc.tile_pool(name="oproj", bufs=1) as opool,
                tc.tile_pool(name="ostage", bufs=4) as ostage,
                tc.tile_pool(name="opsum", bufs=4, space="PSUM") as opsum,
            ):
                wo_sb = opool.tile([128, 8 * D], f32r)
                nc.sync.dma_start(wo_sb[:], wo.rearrange("(t p) o -> p t o", p=128).bitcast(f32r))
                ao_sb = opool.tile([128, 8 * R_OUT], f32r)
                nc.sync.dma_start(
                    ao_sb[:], a2a_out.rearrange("(t p) r -> p t r", p=128).bitcast(f32r))

                for rt in range(R_OUT // 128):
                    for oc in range(2):
                        ops = opsum.tile([128, 512], f32, tag="ops")
                        for ct in range(8):
                            nc.tensor.matmul(
                                ops[:],
                                lhsT=ao_sb[:, ct * R_OUT + rt * 128:
                                              ct * R_OUT + (rt + 1) * 128],
                                rhs=wo_sb[:, ct * D + oc * 512:
                                             ct * D + (oc + 1) * 512],
                                start=(ct == 0), stop=False,
                            )
                        # bias via K=1 ones row
                        nc.tensor.matmul(
                            ops[:],
                            lhsT=ones1[:],
                            rhs=bo_sb[:, oc * 512:(oc + 1) * 512],
                            start=False, stop=True,
                        )
                        o_sb = ostage.tile([128, 512], f32, tag="osb")
                        nc.vector.tensor_copy(o_sb[:], ops[:])
                        nc.sync.dma_start(
                            y[rt * 128:(rt + 1) * 128, oc * 512:(oc + 1) * 512],
                            o_sb[:])

    nc.compile()
    return nc


def kernel(x, wq, bq, wk, bk, wv, bv, wo, bo):
    global _LAST_RESULTS
    x = np.ascontiguousarray(np.asarray(x, dtype=np.float32)).reshape(ROWS, D)
    wo_f = np.ascontiguousarray(np.asarray(wo, dtype=np.float32))
    bo_f = np.ascontiguousarray(np.asarray(bo, dtype=np.float32)).reshape(1, D)

    in_maps = []
    for c in range(N_CORES):
        sl = slice(c * HD, (c + 1) * HD)
        in_maps.append({
            "x": x,
            "wq": np.ascontiguousarray(np.asarray(wq, np.float32)[:, sl]),
            "wk": np.ascontiguousarray(np.asarray(wk, np.float32)[:, sl]),
            "wv": np.ascontiguousarray(np.asarray(wv, np.float32)[:, sl]),
            "bq": np.ascontiguousarray(np.asarray(bq, np.float32)[sl].reshape(HD, 1)),
            "bk": np.ascontiguousarray(np.asarray(bk, np.float32)[sl].reshape(HD, 1)),
            "bv": np.ascontiguousarray(np.asarray(bv, np.float32)[sl].reshape(HD, 1)),
            "wo": wo_f,
            "bo": bo_f,
        })

    nc = build_program()
    res = bass_utils.run_bass_kernel_spmd(nc, in_maps, core_ids=list(range(N_CORES)))
    _LAST_RESULTS = res
    out = np.concatenate([res.results[c]["y"] for c in range(N_CORES)], axis=0)
    return out.reshape(B, NSEQ, D)
